# revision 5
# baseline (speedup 1.0000x reference)
"""Trainium2 Bass kernel: discounted episode returns + normalization.

reference math (full [B, T] = [4096, 8192] f32 inputs):
    ret[t] = rew[t] + 0.99 * ret[t+1] * (1 - done[t])      (reverse-time scan)
    out = (ret - ret.mean()) / (ret.std(axis=-1, ddof=1, keepdims=True) + 1e-9)

Sharding: batch axis split across 8 NeuronCores (512 rows each). The scan is
data-parallel over batch; the global mean needs one scalar AllReduce.

On-core mapping: the recurrence is DVE tensor_tensor_scan
(state = a[t]*state + rew[t], a = 0.99*(1-done)) over negative-stride
(time-reversed) APs; returns stay resident in SBUF so HBM traffic is the
roofline-minimal read(rew)+read(done)+write(out).

Engine balance (v3): DVE = a-coefficients (tensor_scalar 2x) + scan +
normalize; ACT = Square+accum and Copy+accum row stats; PE accumulates the
cross-partition partial sum in PSUM; GpSimd idle (shares an SBUF port with
DVE — keeping it quiet keeps the scan at full rate). 1/(std+eps) is computed
during the AllReduce wait; the AR result is partition-broadcast by DMA.
"""

from contextlib import ExitStack

import numpy as np

import concourse.bass as bass
import concourse.mybir as mybir
import concourse.tile as tile
from concourse import bacc
from concourse.bass_utils import run_bass_kernel_spmd

F32 = mybir.dt.float32
Alu = mybir.AluOpType
Act = mybir.ActivationFunctionType
AxL = mybir.AxisListType

DISCOUNT = 0.99
EPS = 1e-9
P = 128

N_CORES = 8
B_GLOBAL, T = 4096, 8192
B_CORE = B_GLOBAL // N_CORES
CHUNK = 2048


def _build_core_program(tc, out_ap, rew_ap, done_ap, n_cores, total_elems,
                        chunk=CHUNK, out_chunk=None):
    nc = tc.nc
    B_core, T_ = rew_ap.shape
    n_blocks = B_core // P
    n_chunks = T_ // chunk
    out_chunk = out_chunk or chunk

    with ExitStack() as ctx:
        ret_pool = ctx.enter_context(tc.tile_pool(name="ret", bufs=1))
        rew_pool = ctx.enter_context(tc.tile_pool(name="rew", bufs=3))
        done_pool = ctx.enter_context(tc.tile_pool(name="done", bufs=3))
        stat_pool = ctx.enter_context(tc.tile_pool(name="stat", bufs=1))
        psum_pool = ctx.enter_context(tc.tile_pool(name="psum", bufs=1, space="PSUM"))
        dram_pool = ctx.enter_context(tc.tile_pool(name="dram", bufs=1, space="DRAM"))

        ones_col = stat_pool.tile([P, 1], F32)
        nc.vector.memset(ones_col[:], 1.0)

        sum_cat = stat_pool.tile([P, n_blocks], F32)  # col b = row sums of block b
        ss_cat = stat_pool.tile([P, n_blocks], F32)   # col b = row sums of squares
        psum_s = psum_pool.tile([1, 1], F32, tag="psum_s", name="psum_s")

        ret_tiles = []
        for b in range(n_blocks):
            rows = slice(b * P, (b + 1) * P)
            ret_t = ret_pool.tile([P, T_], F32, tag=f"ret{b}", name=f"ret{b}")
            ret_tiles.append(ret_t)
            ss_parts = stat_pool.tile([P, n_chunks], F32, tag=f"ssp{b}",
                                      name=f"ssp{b}")
            sum_parts = stat_pool.tile([P, n_chunks], F32, tag=f"smp{b}",
                                       name=f"smp{b}")
            for ci in range(n_chunks - 1, -1, -1):  # reverse time order
                lo, hi = ci * chunk, (ci + 1) * chunk
                rew_t = rew_pool.tile([P, chunk], F32, tag="rew", name="rew_t")
                nc.sync.dma_start(rew_t[:], rew_ap[rows, lo:hi])
                done_t = done_pool.tile([P, chunk], F32, tag="done", name="done_t")
                nc.sync.dma_start(done_t[:], done_ap[rows, lo:hi])
                # a = 0.99 - 0.99*done, in place (exact for done in {0,1}).
                # Alternate DVE/ACT so neither engine's per-chunk work
                # exceeds the chunk's DMA wire time.
                if ci % 2 == 0:
                    nc.vector.tensor_scalar(done_t[:], done_t[:], -DISCOUNT,
                                            DISCOUNT, Alu.mult, Alu.add)
                else:
                    nc.scalar.activation(done_t[:], done_t[:], Act.Copy,
                                         bias=DISCOUNT, scale=-DISCOUNT)
                # reversed scan: state = a*state + rew, columns hi-1 .. lo
                init = 0.0 if ci == n_chunks - 1 else ret_t[:, hi:hi + 1]
                nc.vector.tensor_tensor_scan(
                    ret_t[:, lo:hi][:, ::-1], done_t[:, ::-1], rew_t[:, ::-1],
                    init, Alu.mult, Alu.add)
                # per-chunk row stats on ACT; done_t is dead -> reuse as scratch
                nc.scalar.activation(done_t[:], ret_t[:, lo:hi], Act.Square,
                                     accum_out=ss_parts[:, ci:ci + 1])
                nc.scalar.activation(done_t[:], ret_t[:, lo:hi], Act.Copy,
                                     accum_out=sum_parts[:, ci:ci + 1])
            nc.vector.tensor_reduce(sum_cat[:, b:b + 1], sum_parts[:], AxL.X,
                                    Alu.add)
            nc.vector.tensor_reduce(ss_cat[:, b:b + 1], ss_parts[:], AxL.X, Alu.add)
            # accumulate this block's cross-partition total into PSUM
            nc.tensor.matmul(psum_s[:], ones_col[:], sum_cat[:, b:b + 1],
                             start=(b == 0), stop=(b == n_blocks - 1))

        # ---- per-row 1/(std+eps): independent of the AllReduce, overlaps it ----
        sum_sq = stat_pool.tile([P, n_blocks], F32)
        nc.vector.tensor_tensor(sum_sq[:], sum_cat[:], sum_cat[:], Alu.mult)
        u = stat_pool.tile([P, n_blocks], F32)
        nc.vector.scalar_tensor_tensor(u[:], sum_sq[:], -1.0 / T_, ss_cat[:],
                                       Alu.mult, Alu.add)  # ss - sum^2/T
        stdv = stat_pool.tile([P, n_blocks], F32)
        nc.scalar.activation(stdv[:], u[:], Act.Sqrt, scale=1.0 / (T_ - 1))
        nc.vector.tensor_scalar_add(stdv[:], stdv[:], EPS)
        inv_cat = stat_pool.tile([P, n_blocks], F32)
        nc.vector.reciprocal(inv_cat[:], stdv[:])

        # ---- global mean: PSUM total -> scalar AllReduce -> broadcast DMA ----
        s11 = stat_pool.tile([1, 1], F32)
        nc.vector.tensor_copy(s11[:], psum_s[:])
        gsum_b = stat_pool.tile([P, 1], F32)
        if n_cores > 1:
            ar_in = dram_pool.tile([1, 1], F32, tag="ar_in", name="ar_in")
            ar_out = dram_pool.tile([1, 1], F32, tag="ar_out", name="ar_out")
            nc.sync.dma_start(ar_in[:], s11[:])
            nc.gpsimd.collective_compute(
                "AllReduce", Alu.add,
                replica_groups=[list(range(n_cores))],
                ins=[ar_in.opt()], outs=[ar_out.opt()])
            # gpsimd holds the AR completion; issuing the broadcast from it
            # saves a cross-engine hop on the critical path
            nc.gpsimd.dma_start(gsum_b[:], ar_out[:].partition_broadcast(P))
        else:
            loc = dram_pool.tile([1, 1], F32, tag="loc", name="loc")
            nc.sync.dma_start(loc[:], s11[:])
            nc.sync.dma_start(gsum_b[:], loc[:].partition_broadcast(P))

        negb_cat = stat_pool.tile([P, n_blocks], F32)
        nc.vector.tensor_scalar(negb_cat[:], inv_cat[:], gsum_b[:, 0:1],
                                -1.0 / total_elems, Alu.mult, Alu.mult)

        # ---- normalize in place on DVE, stream out per chunk ----
        for b in range(n_blocks):
            rows = slice(b * P, (b + 1) * P)
            ret_t = ret_tiles[b]
            for ci in range(T_ // out_chunk):
                lo, hi = ci * out_chunk, (ci + 1) * out_chunk
                nc.vector.tensor_scalar(ret_t[:, lo:hi], ret_t[:, lo:hi],
                                        inv_cat[:, b:b + 1], negb_cat[:, b:b + 1],
                                        Alu.mult, Alu.add)
                nc.sync.dma_start(out_ap[rows, lo:hi], ret_t[:, lo:hi])


_NC_CACHE = None


def _get_nc():
    global _NC_CACHE
    if _NC_CACHE is None:
        nc = bacc.Bacc("TRN2", target_bir_lowering=False, debug=False,
                       enable_asserts=False, num_devices=N_CORES)
        rew = nc.dram_tensor("rewards", [B_CORE, T], F32, kind="ExternalInput")
        done = nc.dram_tensor("done_flags", [B_CORE, T], F32, kind="ExternalInput")
        out = nc.dram_tensor("out", [B_CORE, T], F32, kind="ExternalOutput")
        with tile.TileContext(nc) as tc:
            _build_core_program(tc, out.ap(), rew.ap(), done.ap(),
                                n_cores=N_CORES, total_elems=B_GLOBAL * T)
        nc.compile()
        _NC_CACHE = nc
    return _NC_CACHE


def run_sharded(rewards, done_flags, trace=False, **kwargs):
    """Run the SPMD kernel; returns (full_output, BassKernelResults)."""
    nc = _get_nc()
    in_maps = []
    for c in range(N_CORES):
        rows = slice(c * B_CORE, (c + 1) * B_CORE)
        in_maps.append({
            "rewards": np.ascontiguousarray(rewards[rows]),
            "done_flags": np.ascontiguousarray(done_flags[rows]),
        })
    res = run_bass_kernel_spmd(nc, in_maps, core_ids=list(range(N_CORES)),
                               trace=trace, **kwargs)
    full = np.concatenate([res.results[c]["out"] for c in range(N_CORES)], axis=0)
    return full, res


def kernel(rewards, done_flags):
    out, _ = run_sharded(rewards, done_flags, trace=False)
    return out


# revision 6
# speedup vs baseline: 1.1643x; 1.1643x over previous
"""Trainium2 Bass kernel: discounted episode returns + normalization.

reference math (full [B, T] = [4096, 8192] f32 inputs):
    ret[t] = rew[t] + 0.99 * ret[t+1] * (1 - done[t])      (reverse-time scan)
    out = (ret - ret.mean()) / (ret.std(axis=-1, ddof=1, keepdims=True) + 1e-9)

Sharding: batch axis split across 8 NeuronCores (512 rows each). The scan is
data-parallel over batch; the global mean needs one scalar AllReduce.

On-core mapping: the recurrence is DVE tensor_tensor_scan
(state = a[t]*state + rew[t], a = 0.99*(1-done)) over negative-stride
(time-reversed) APs; returns stay resident in SBUF so HBM traffic is the
roofline-minimal read(rew)+read(done)+write(out).

Engine balance (v3): DVE = a-coefficients (tensor_scalar 2x) + scan +
normalize; ACT = Square+accum and Copy+accum row stats; PE accumulates the
cross-partition partial sum in PSUM; GpSimd idle (shares an SBUF port with
DVE — keeping it quiet keeps the scan at full rate). 1/(std+eps) is computed
during the AllReduce wait; the AR result is partition-broadcast by DMA.
"""

from contextlib import ExitStack

import numpy as np

import concourse.bass as bass
import concourse.mybir as mybir
import concourse.tile as tile
from concourse import bacc
from concourse.bass_utils import run_bass_kernel_spmd

F32 = mybir.dt.float32
Alu = mybir.AluOpType
Act = mybir.ActivationFunctionType
AxL = mybir.AxisListType

DISCOUNT = 0.99
EPS = 1e-9
P = 128

N_CORES = 8
B_GLOBAL, T = 4096, 8192
B_CORE = B_GLOBAL // N_CORES
CHUNK = 2048


def _build_core_program(tc, out_ap, rew_ap, done_ap, n_cores, total_elems,
                        chunk=CHUNK, out_chunk=None):
    nc = tc.nc
    B_core, T_ = rew_ap.shape
    n_blocks = B_core // P
    n_chunks = T_ // chunk
    out_chunk = out_chunk or chunk

    with ExitStack() as ctx:
        ret_pool = ctx.enter_context(tc.tile_pool(name="ret", bufs=1))
        rew_pool = ctx.enter_context(tc.tile_pool(name="rew", bufs=3))
        done_pool = ctx.enter_context(tc.tile_pool(name="done", bufs=3))
        stat_pool = ctx.enter_context(tc.tile_pool(name="stat", bufs=1))
        psum_pool = ctx.enter_context(tc.tile_pool(name="psum", bufs=1, space="PSUM"))
        dram_pool = ctx.enter_context(tc.tile_pool(name="dram", bufs=1, space="DRAM"))

        ones_col = stat_pool.tile([P, 1], F32)
        nc.vector.memset(ones_col[:], 1.0)

        sum_cat = stat_pool.tile([P, n_blocks], F32)  # col b = row sums of block b
        ss_cat = stat_pool.tile([P, n_blocks], F32)   # col b = row sums of squares
        psum_s = psum_pool.tile([1, 1], F32, tag="psum_s", name="psum_s")

        ret_tiles = []
        for b in range(n_blocks):
            rows = slice(b * P, (b + 1) * P)
            ret_t = ret_pool.tile([P, T_], F32, tag=f"ret{b}", name=f"ret{b}")
            ret_tiles.append(ret_t)
            ss_parts = stat_pool.tile([P, n_chunks], F32, tag=f"ssp{b}",
                                      name=f"ssp{b}")
            sum_parts = stat_pool.tile([P, n_chunks], F32, tag=f"smp{b}",
                                       name=f"smp{b}")
            for ci in range(n_chunks - 1, -1, -1):  # reverse time order
                lo, hi = ci * chunk, (ci + 1) * chunk
                rew_t = rew_pool.tile([P, chunk], F32, tag="rew", name="rew_t")
                nc.sync.dma_start(rew_t[:], rew_ap[rows, lo:hi])
                done_t = done_pool.tile([P, chunk], F32, tag="done", name="done_t")
                nc.sync.dma_start(done_t[:], done_ap[rows, lo:hi])
                # a = 0.99 - 0.99*done, in place on DVE (exact for done in
                # {0,1}). Keeping a+scan on one engine keeps the serial
                # carry chain free of cross-engine hops.
                nc.vector.tensor_scalar(done_t[:], done_t[:], -DISCOUNT, DISCOUNT,
                                        Alu.mult, Alu.add)
                # reversed scan: state = a*state + rew, columns hi-1 .. lo
                init = 0.0 if ci == n_chunks - 1 else ret_t[:, hi:hi + 1]
                nc.vector.tensor_tensor_scan(
                    ret_t[:, lo:hi][:, ::-1], done_t[:, ::-1], rew_t[:, ::-1],
                    init, Alu.mult, Alu.add)
                # per-chunk row stats on ACT; done_t is dead -> reuse as scratch
                nc.scalar.activation(done_t[:], ret_t[:, lo:hi], Act.Square,
                                     accum_out=ss_parts[:, ci:ci + 1])
                nc.scalar.activation(done_t[:], ret_t[:, lo:hi], Act.Copy,
                                     accum_out=sum_parts[:, ci:ci + 1])
            nc.vector.tensor_reduce(sum_cat[:, b:b + 1], sum_parts[:], AxL.X,
                                    Alu.add)
            nc.vector.tensor_reduce(ss_cat[:, b:b + 1], ss_parts[:], AxL.X, Alu.add)
            # accumulate this block's cross-partition total into PSUM
            nc.tensor.matmul(psum_s[:], ones_col[:], sum_cat[:, b:b + 1],
                             start=(b == 0), stop=(b == n_blocks - 1))

        # ---- per-row 1/(std+eps): independent of the AllReduce, overlaps it ----
        sum_sq = stat_pool.tile([P, n_blocks], F32)
        nc.vector.tensor_tensor(sum_sq[:], sum_cat[:], sum_cat[:], Alu.mult)
        u = stat_pool.tile([P, n_blocks], F32)
        nc.vector.scalar_tensor_tensor(u[:], sum_sq[:], -1.0 / T_, ss_cat[:],
                                       Alu.mult, Alu.add)  # ss - sum^2/T
        stdv = stat_pool.tile([P, n_blocks], F32)
        nc.scalar.activation(stdv[:], u[:], Act.Sqrt, scale=1.0 / (T_ - 1))
        nc.vector.tensor_scalar_add(stdv[:], stdv[:], EPS)
        inv_cat = stat_pool.tile([P, n_blocks], F32)
        nc.vector.reciprocal(inv_cat[:], stdv[:])

        # ---- global mean: PSUM total -> scalar AllReduce -> broadcast DMA ----
        s11 = stat_pool.tile([1, 1], F32)
        nc.vector.tensor_copy(s11[:], psum_s[:])
        gsum_b = stat_pool.tile([P, 1], F32)
        if n_cores > 1:
            ar_in = dram_pool.tile([1, 1], F32, tag="ar_in", name="ar_in")
            ar_out = dram_pool.tile([1, 1], F32, tag="ar_out", name="ar_out")
            nc.sync.dma_start(ar_in[:], s11[:])
            nc.gpsimd.collective_compute(
                "AllReduce", Alu.add,
                replica_groups=[list(range(n_cores))],
                ins=[ar_in.opt()], outs=[ar_out.opt()])
            # gpsimd holds the AR completion; issuing the broadcast from it
            # saves a cross-engine hop on the critical path
            nc.gpsimd.dma_start(gsum_b[:], ar_out[:].partition_broadcast(P))
        else:
            loc = dram_pool.tile([1, 1], F32, tag="loc", name="loc")
            nc.sync.dma_start(loc[:], s11[:])
            nc.sync.dma_start(gsum_b[:], loc[:].partition_broadcast(P))

        negb_cat = stat_pool.tile([P, n_blocks], F32)
        nc.vector.tensor_scalar(negb_cat[:], inv_cat[:], gsum_b[:, 0:1],
                                -1.0 / total_elems, Alu.mult, Alu.mult)

        # ---- normalize in place on DVE, stream out per chunk ----
        for b in range(n_blocks):
            rows = slice(b * P, (b + 1) * P)
            ret_t = ret_tiles[b]
            for ci in range(T_ // out_chunk):
                lo, hi = ci * out_chunk, (ci + 1) * out_chunk
                nc.vector.tensor_scalar(ret_t[:, lo:hi], ret_t[:, lo:hi],
                                        inv_cat[:, b:b + 1], negb_cat[:, b:b + 1],
                                        Alu.mult, Alu.add)
                nc.sync.dma_start(out_ap[rows, lo:hi], ret_t[:, lo:hi])


_NC_CACHE = None


def _get_nc():
    global _NC_CACHE
    if _NC_CACHE is None:
        nc = bacc.Bacc("TRN2", target_bir_lowering=False, debug=False,
                       enable_asserts=False, num_devices=N_CORES)
        rew = nc.dram_tensor("rewards", [B_CORE, T], F32, kind="ExternalInput")
        done = nc.dram_tensor("done_flags", [B_CORE, T], F32, kind="ExternalInput")
        out = nc.dram_tensor("out", [B_CORE, T], F32, kind="ExternalOutput")
        with tile.TileContext(nc) as tc:
            _build_core_program(tc, out.ap(), rew.ap(), done.ap(),
                                n_cores=N_CORES, total_elems=B_GLOBAL * T)
        nc.compile()
        _NC_CACHE = nc
    return _NC_CACHE


def run_sharded(rewards, done_flags, trace=False, **kwargs):
    """Run the SPMD kernel; returns (full_output, BassKernelResults)."""
    nc = _get_nc()
    in_maps = []
    for c in range(N_CORES):
        rows = slice(c * B_CORE, (c + 1) * B_CORE)
        in_maps.append({
            "rewards": np.ascontiguousarray(rewards[rows]),
            "done_flags": np.ascontiguousarray(done_flags[rows]),
        })
    res = run_bass_kernel_spmd(nc, in_maps, core_ids=list(range(N_CORES)),
                               trace=trace, **kwargs)
    full = np.concatenate([res.results[c]["out"] for c in range(N_CORES)], axis=0)
    return full, res


def kernel(rewards, done_flags):
    out, _ = run_sharded(rewards, done_flags, trace=False)
    return out
